# revision 23
# baseline (speedup 1.0000x reference)
"""GATv2 (2-layer) + global mean pool on 8 Trainium2 NeuronCores.

Strategy (per core): nodes are partitioned into 8 contiguous blocks of 6250
dst nodes.  Each core processes all edges whose dst lands in its block.
Edges are host-sorted by (src-chunk, dst-block-of-128) and padded so every
(chunk, block) group has a fixed number of 128-edge subtiles -> the Bass
instruction stream is identical on all 8 cores (SPMD), only data differs.

Device per layer (bound by Q7/SWDGE gather descriptor generation at
~8ns/edge; everything else overlaps under it):
  - el = table[src] via dma_gather from a bf16 table whose rows are padded
    to 128 cols so each descriptor is the 256B minimum.  One gather call per
    (chunk, dst-block) group; pad slots point at a zero dummy row.  (The
    num_idxs_reg / trailing -1 trim variants that would skip pad descriptors
    are implemented behind GAT_CNT_REG / GAT_NEG_TRIM but default OFF: on
    HW both wedge the device, likely a DMA-semaphore hang when an SDMA
    engine receives no descriptors.)
  - self-loop edges never enter the gather: per block they are one subtile
    whose el/xr are direct reads of small resident tables and whose scatter
    one-hot is the identity; all 49 are batched into a handful of DVE ops
    emitted before the edge sweep (they fill the desc-gen startup bubble and
    carry the PSUM bank start=True flags).
  - |att| is folded into the xl/xr tables on the host with channels permuted
    into exact (head, sign-of-att) groups: the score becomes
    sum_c lrelu(z')[group +] - sum_c lrelu(z')[group -] (one small reduce
    per group), killing the per-edge r*att multiply on DVE.  The fold is
    undone in the per-bank normalize by a per-channel 1/|att| multiply (and
    the host un-permutes h_out columns).
  - xr_d expansion on the tensor engine: onehotT matmul'd with the 128-row
    xr block (xrt stays resident in SBUF) -> PSUM, drained by ACT.
  - bf16 edge math on DVE: z = el + xr_d ; r = leaky_relu(z); score =
    signed group reduce; alpha' = exp(score) written by ACT directly into
    the scatter payload ev[:, C':C'+h].
  - w = alpha' * el (bf16), scattered to dst via one-hot matmuls
    accumulated in PSUM (49 block accumulators packed 7-per-bank; start=True
    only on each bank's first matmul = its first self-loop subtile), then
    h = (num/(den+1e-16)) * inv|att| + b', ELU.  Each bank is normalized as
    soon as its last block stops accumulating.
Host does the projections (x@W) + the att fold/permute, the gather of h
between layers, the global mean pool, and the tiny [64,2] head.
"""

import os
import sys

import numpy as np


def _setup_paths():
    for p in (
        "/opt/trn_rl_repo",
        os.path.expanduser("~/.axon_site/_ro/trn_rl_repo"),
    ):
        if os.path.isdir(p) and p not in sys.path:
            sys.path.insert(0, p)


_setup_paths()

import concourse.bacc as bacc  # noqa: E402
import concourse.bass as bass  # noqa: E402
import concourse.mybir as mybir  # noqa: E402
import concourse.tile as tile  # noqa: E402

AF = mybir.ActivationFunctionType
ALU = mybir.AluOpType
F32 = mybir.dt.float32
I16 = mybir.dt.int16
I32 = mybir.dt.int32

# ---------------------------------------------------------------- model dims
N_NODES = 50000
N_EDGES = 1600000
N_GRAPHS = 64
DIM_IN, DIM_H, HEADS, DIM_OUT = 128, 32, 2, 64
NEG_SLOPE = 0.2

N_CORES = 8
USE_CNT_REG = os.environ.get("GAT_CNT_REG", "0") == "1"
NEG_TRIM = os.environ.get("GAT_NEG_TRIM", "0") == "1"
P = 128          # partitions / edges per subtile
BLK = 128        # dst nodes per block
JT = 16          # subtiles per compute tile
JCALL = 48       # max subtiles per gather call
GRP = 4
CHUNK = 32767    # src-index chunk size (int16 limit)
ATT_EPS = 1e-6


def cdiv(a, b):
    return (a + b - 1) // b


# ============================================================== host prep
def pack_idx(v, pad_to_cols=None):
    """Pack an int array into the dma_gather idx layout:
    idx i -> partition i%16 (replicated to all 8 groups of 16), col i//16."""
    n = len(v)
    assert n % 16 == 0
    cols = n // 16
    a = np.asarray(v, np.int16).reshape(cols, 16).T  # [16, cols]
    a = np.tile(a, (8, 1))  # [128, cols]
    if pad_to_cols is not None and cols < pad_to_cols:
        a = np.pad(a, ((0, 0), (0, pad_to_cols - cols)), constant_values=-1)
    return a


def fold_att(att, heads, ch):
    """Channel permutation + scale for the att fold (no padding).

    Returns (perm, scale, glens): perm[slot] = source channel, scale[slot] =
    |att[src]| clamped, slots laid out as [h0+, h0-, h1+, h1-, ...] with
    exact group lengths glens[2h] (so each head's ch columns stay
    contiguous and C2 == heads*ch)."""
    a = np.asarray(att, np.float64).reshape(heads, ch)
    perm, scale, glens = [], [], []
    for h in range(heads):
        pos = np.where(a[h] >= 0)[0]
        neg = np.where(a[h] < 0)[0]
        for idxs in (pos, neg):
            glens.append(len(idxs))
            for c in idxs:
                perm.append(h * ch + c)
                scale.append(max(abs(a[h, c]), ATT_EPS))
    return np.asarray(perm, np.int64), np.asarray(scale, np.float64), glens


class GraphPlan:
    """Core-uniform edge layout shared by both layers (self-loops excluded
    from the gather; handled as identity subtiles)."""

    def __init__(self, edge_index, n_nodes, n_cores, chunk=CHUNK):
        src = np.asarray(edge_index[0], np.int64)
        dst = np.asarray(edge_index[1], np.int64)
        self.n_nodes = n_nodes
        self.n_cores = n_cores
        self.chunk = chunk
        npc = n_nodes // n_cores
        self.npc = npc
        nblk = cdiv(npc, BLK)
        self.nblk = nblk
        self.rows0 = min(chunk, n_nodes) + 1   # chunk0 src rows + dummy
        self.rows1 = max(n_nodes - chunk, 0) + 1  # chunk1 src rows + dummy
        self.xr_rows = nblk * BLK              # block rows (pad rows zero)

        core = dst // npc
        per = {}
        for c in range(n_cores):
            m = core == c
            s, d = src[m], dst[m]
            dloc = d - c * npc
            b = dloc // BLK
            ch = (s >= chunk).astype(np.int64)
            key = ch * nblk + b
            order = np.argsort(key, kind="stable")
            s, dloc, key = s[order], dloc[order], key[order]
            per[c] = (s, dloc, key)
        # per-(chunk, block) subtile counts: max over cores (static shapes)
        Sb = np.ones((2, nblk), np.int64)
        cnts = np.zeros((n_cores, 2, nblk), np.int64)
        for c in range(n_cores):
            _, _, key = per[c]
            cnt = np.bincount(key, minlength=2 * nblk)
            cnts[c] = cnt.reshape(2, nblk)
            for ch in (0, 1):
                v = np.maximum(1, -(-cnt[ch * nblk : (ch + 1) * nblk] // P))
                Sb[ch] = np.maximum(Sb[ch], v)
        self.Sb = Sb
        self.cnts = cnts  # real edge count per (core, chunk, block)
        self.n_sub_total = int(Sb.sum())
        self.Smax = int(Sb.max())

        # subtile -> (block, stop) ; uniform across cores.  start flags live
        # on the self-loop subtiles now.
        self.sub_block = []
        self.sub_stop = []
        for ch in (0, 1):
            for b in range(nblk):
                nsb = int(Sb[ch][b])
                for j in range(nsb):
                    self.sub_block.append(b)
                    self.sub_stop.append(ch == 1 and j == nsb - 1)

        # gather calls: slabs of <= JCALL subtiles per chunk, greedily
        # packing whole (chunk, block) groups (pads stay dummy-row descs,
        # so calls may span block boundaries freely).
        self.calls = []  # (chunk, n_subtiles)
        for ch in (0, 1):
            acc = 0
            for b in range(nblk):
                take = int(Sb[ch][b])
                if acc and acc + take > JCALL:
                    self.calls.append((ch, acc))
                    acc = 0
                acc += take
            if acc:
                self.calls.append((ch, acc))
        self.n_calls = len(self.calls)

        # compute tiles: JT-slices within each call
        self.tiles = []  # (call_idx, j0, J2, sub_base)
        sub_base = 0
        for ci, (ch, nsb) in enumerate(self.calls):
            j0 = 0
            while j0 < nsb:
                J2 = min(JT, nsb - j0)
                self.tiles.append((ci, j0, J2, sub_base + j0))
                j0 += J2
            sub_base += nsb
        self.n_tiles = len(self.tiles)

        # per-core slot data
        import ml_dtypes

        BF = ml_dtypes.bfloat16
        self.core_el_idx = []   # [n_calls, 128, JCALL*8] int16
        self.core_cnt = []      # [1, n_calls] int32 (real edges per call)
        self.core_ds = []       # [n_tiles, 128, JT] bf16 (edge-major)
        self.core_dsf = []      # [n_tiles, 1, JT*128] bf16 (free-major)
        dummy0 = self.rows0 - 1
        dummy1 = self.rows1 - 1
        # padded-subtile start of each (chunk, block) group, in group order
        grp_start = {}
        off = 0
        for ch in (0, 1):
            for b in range(nblk):
                grp_start[(ch, b)] = off
                off += int(Sb[ch][b])
        call_start = {}
        off = 0
        for ci, (ch, nsb) in enumerate(self.calls):
            call_start[ci] = off
            off += nsb
        for c in range(n_cores):
            s, dloc, key = per[c]
            el_slots = np.full(self.n_sub_total * P, 0, np.int32)
            ds_slots = np.full(self.n_sub_total * P, 240.0, np.float32)
            cnt = np.bincount(key, minlength=2 * nblk)
            starts = np.concatenate([[0], np.cumsum(cnt)[:-1]])
            cnt_arr = np.zeros((1, self.n_calls), np.int32)
            for ch in (0, 1):
                for b in range(nblk):
                    k = ch * nblk + b
                    n = int(cnt[k])
                    a0 = int(starts[k])
                    off = grp_start[(ch, b)] * P
                    sl = slice(off, off + n)
                    ss = s[a0 : a0 + n]
                    el_slots[sl] = ss - (chunk if ch else 0)
                    ds_slots[sl] = (dloc[a0 : a0 + n] % BLK).astype(np.float32)
                    # pad slots point at the zero dummy row (GAT_NEG_TRIM=1
                    # trailing -1s wedge the device; see module docstring)
                    pad = slice(off + n, off + int(Sb[ch][b]) * P)
                    el_slots[pad] = (
                        -1 if NEG_TRIM else (dummy1 if ch else dummy0)
                    )
            self.core_cnt.append(cnt_arr)
            el_idx = np.full((self.n_calls, 128, JCALL * 8), -1, np.int16)
            for ci, (ch, nsb) in enumerate(self.calls):
                n = nsb * P
                base = call_start[ci] * P
                el_idx[ci, :, : n // 16] = pack_idx(el_slots[base : base + n])
            self.core_el_idx.append(el_idx)
            ds = np.full((self.n_tiles, 128, JT), 240.0, np.float32)
            dsf = np.full((self.n_tiles, 1, JT * 128), 240.0, np.float32)
            for ti, (ci, j0, J2, sb) in enumerate(self.tiles):
                blkv = ds_slots[sb * P : (sb + J2) * P].reshape(J2, P)
                ds[ti, :, :J2] = blkv.T
                dsf[ti, 0, : J2 * 128] = blkv.reshape(-1)
            self.core_ds.append(ds.astype(BF))
            self.core_dsf.append(dsf.astype(BF))

    def make_tables(self, xl2, xr2, core):
        """el chunk tables (bf16, rows padded to 128 cols), xr block table
        and own-range xl table for one core.  xl2/xr2 are the FOLDED
        [n_nodes, C2] node features."""
        import ml_dtypes

        BF = ml_dtypes.bfloat16
        C2 = xl2.shape[1]
        ck = self.chunk
        t0 = np.zeros((self.rows0, 128), BF)
        t0[: min(ck, self.n_nodes), :C2] = xl2[:ck].astype(BF)
        t1 = np.zeros((self.rows1, 128), BF)
        t1[: max(self.n_nodes - ck, 0), :C2] = xl2[ck:].astype(BF)
        lo = core * self.npc
        hi = lo + self.npc
        xrt = np.zeros((self.xr_rows, C2), np.float32)
        xrt[: self.npc] = xr2[lo:hi]
        xlo = np.zeros((self.xr_rows, C2), np.float32)
        xlo[: self.npc] = xl2[lo:hi]
        # rearrange to [128, nblk*C2] (partition-major blocks)
        xrt_r = xrt.reshape(self.nblk, BLK, C2).transpose(1, 0, 2).reshape(BLK, -1)
        xlo_r = xlo.reshape(self.nblk, BLK, C2).transpose(1, 0, 2).reshape(BLK, -1)
        return t0, t1, xrt_r.astype(BF), xlo_r.astype(BF)


# ============================================================== bass builder
def build_layer(plan: GraphPlan, heads, ch, glens, do_pool, n_graphs=N_GRAPHS):
    """One GATv2 layer for one core (SPMD across 8).  glens: the 2h exact
    (head, sign) group lengths of the permuted channel layout."""
    BF16 = mybir.dt.bfloat16
    C = heads * ch               # channel count (permuted, unpadded)
    W = C + heads                # psum accum row: [w-sums | exp-sums]
    assert sum(glens) == C and len(glens) == 2 * heads
    goff = np.concatenate([[0], np.cumsum(glens)]).astype(int)
    nblk = plan.nblk
    per_bank = 7
    n_banks = cdiv(nblk, per_bank)
    assert per_bank * W <= 512 and n_banks <= 7, (W, n_banks)
    EXG = 8

    nc = bacc.Bacc()
    elt0 = nc.dram_tensor("elt0", [plan.rows0, 128], BF16, kind="ExternalInput")
    elt1 = nc.dram_tensor("elt1", [plan.rows1, 128], BF16, kind="ExternalInput")
    xrt_d = nc.dram_tensor("xrt", [128, nblk * C], BF16, kind="ExternalInput")
    xlo_d = nc.dram_tensor("xlo", [128, nblk * C], BF16, kind="ExternalInput")
    iota_d = nc.dram_tensor("iota128", [128, 128], BF16, kind="ExternalInput")
    ident_d = nc.dram_tensor("ident", [128, 128], BF16, kind="ExternalInput")
    iotapr_d = nc.dram_tensor(
        "iota_pr", [128, JT * 128], BF16, kind="ExternalInput"
    )
    b_d = nc.dram_tensor("b_rep", [128, C], F32, kind="ExternalInput")
    ia_d = nc.dram_tensor("inv_att", [128, C], F32, kind="ExternalInput")
    idx_el_d = nc.dram_tensor(
        "idx_el", [plan.n_calls, 128, JCALL * 8], I16, kind="ExternalInput"
    )
    cnt_d = nc.dram_tensor("cnt", [1, plan.n_calls], I32, kind="ExternalInput")
    ds_d = nc.dram_tensor("ds", [plan.n_tiles, 128, JT], BF16, kind="ExternalInput")
    dsf_d = nc.dram_tensor(
        "dsf", [plan.n_tiles, 1, JT * 128], BF16, kind="ExternalInput"
    )
    h_out = nc.dram_tensor("h_out", [nblk * BLK, C], F32, kind="ExternalOutput")

    from contextlib import ExitStack

    with tile.TileContext(nc) as tc, ExitStack() as ctx:
        cpool = ctx.enter_context(tc.tile_pool(name="consts", bufs=1))
        gpool = ctx.enter_context(tc.tile_pool(name="gather", bufs=4))
        ipool = ctx.enter_context(tc.tile_pool(name="idx", bufs=12))
        wpool = ctx.enter_context(tc.tile_pool(name="work", bufs=3))
        npool = ctx.enter_context(tc.tile_pool(name="norm", bufs=2))
        pspool = ctx.enter_context(tc.tile_pool(name="ps", bufs=1, space="PSUM"))
        expool = ctx.enter_context(tc.tile_pool(name="exps", bufs=1, space="PSUM"))

        iota_t = cpool.tile([128, 128], BF16, tag="iota")
        nc.sync.dma_start(iota_t[:], iota_d[:, :])
        ident_t = cpool.tile([128, 128], BF16, tag="ident")
        nc.sync.dma_start(ident_t[:], ident_d[:, :])
        iotapr_t = cpool.tile([128, JT * 128], BF16, tag="iotapr")
        nc.sync.dma_start(iotapr_t[:], iotapr_d[:, :])
        b_t = cpool.tile([128, C], F32, tag="bias")
        nc.sync.dma_start(b_t[:], b_d[:, :])
        ia_t = cpool.tile([128, C], F32, tag="invatt")
        nc.sync.dma_start(ia_t[:], ia_d[:, :])
        xrt_t = cpool.tile([128, nblk, C], BF16, tag="xrt")
        nc.sync.dma_start(
            xrt_t[:], xrt_d[:, :].rearrange("p (n c) -> p n c", c=C)
        )
        xlo_t = cpool.tile([128, nblk, C], BF16, tag="xlo")
        nc.sync.dma_start(
            xlo_t[:], xlo_d[:, :].rearrange("p (n c) -> p n c", c=C)
        )
        cnt_t = cpool.tile([1, plan.n_calls], I32, tag="cnt")
        nc.sync.dma_start(cnt_t[:], cnt_d[:, :])

        pacc = [
            pspool.tile([128, per_bank * W], F32, tag=f"pacc{k}", name=f"pacc{k}")
            for k in range(n_banks)
        ]

        # ---------------- normalize + ELU (per bank)
        def do_norm(k):
            nb = min(per_bank, nblk - k * per_bank)
            acc = pacc[k][:, : nb * W].rearrange("p (n w) -> p n w", w=W)
            den_t = npool.tile([128, per_bank * heads], F32, tag="den")
            den = den_t[:, : nb * heads].rearrange("p (n h) -> p n h", h=heads)
            nc.vector.tensor_scalar(den, acc[:, :, C : C + heads], 1e-16, None, ALU.add)
            rec_t = npool.tile([128, per_bank * heads], F32, tag="rec")
            rec = rec_t[:, : nb * heads].rearrange("p (n h) -> p n h", h=heads)
            nc.vector.reciprocal(rec, den)
            h_t = npool.tile([128, per_bank * C], F32, tag="h")
            hv = h_t[:, : nb * C].rearrange("p (n c) -> p n c", c=C)
            nc.vector.tensor_tensor(
                out=hv.rearrange("p n (h c) -> p n h c", h=heads),
                in0=acc[:, :, :C].rearrange("p n (h c) -> p n h c", h=heads),
                in1=rec.unsqueeze(3).broadcast_to([128, nb, heads, ch]),
                op=ALU.mult,
            )
            # undo the |att| fold, then + bias
            nc.vector.tensor_tensor(
                out=hv,
                in0=hv,
                in1=ia_t[:].unsqueeze(1).broadcast_to([128, nb, C]),
                op=ALU.mult,
            )
            nc.vector.tensor_tensor(
                out=hv,
                in0=hv,
                in1=b_t[:].unsqueeze(1).broadcast_to([128, nb, C]),
                op=ALU.add,
            )
            # ELU = relu(x) + exp(min(x,0)) - 1
            re_t = npool.tile([128, per_bank * C], F32, tag="re")
            nc.vector.tensor_scalar_max(re_t[:, : nb * C], h_t[:, : nb * C], 0.0)
            mn_t = npool.tile([128, per_bank * C], F32, tag="mn")
            nc.vector.tensor_scalar_min(mn_t[:, : nb * C], h_t[:, : nb * C], 0.0)
            nc.scalar.activation(mn_t[:, : nb * C], mn_t[:, : nb * C], AF.Exp)
            nc.vector.tensor_tensor(
                out=h_t[:, : nb * C],
                in0=re_t[:, : nb * C],
                in1=mn_t[:, : nb * C],
                op=ALU.add,
            )
            nc.vector.tensor_scalar_add(h_t[:, : nb * C], h_t[:, : nb * C], -1.0)
            for i in range(nb):
                b = k * per_bank + i
                nc.sync.dma_start(h_out[b * BLK : (b + 1) * BLK, :], hv[:, i, :])

        # ---------------- edge-math helper (shared by self-loop + gather)
        def edge_math(el, xrd, J2, ev_t):
            """el, xrd: [128, J2, C] bf16 views.  Fills ev_t[:, :J2, :W]."""
            z_t = wpool.tile([128, JT, C], BF16, tag="z")
            nc.vector.tensor_tensor(
                out=z_t[:, :J2, :], in0=el, in1=xrd, op=ALU.add
            )
            r_t = wpool.tile([128, JT, C], BF16, tag="r")
            nc.vector.scalar_tensor_tensor(
                out=r_t[:, :J2, :],
                in0=z_t[:, :J2, :],
                scalar=NEG_SLOPE,
                in1=z_t[:, :J2, :],
                op0=ALU.mult,
                op1=ALU.max,
            )
            # signed group reduce: exact per-(head, sign) partial sums
            sc_t = wpool.tile([128, JT, 2 * heads], F32, tag="sc")
            for g in range(2 * heads):
                if glens[g] == 0:
                    nc.vector.memset(sc_t[:, :J2, g : g + 1], 0.0)
                    continue
                nc.vector.reduce_sum(
                    out=sc_t[:, :J2, g : g + 1],
                    in_=r_t[:, :J2, goff[g] : goff[g + 1]].unsqueeze(2),
                    axis=mybir.AxisListType.X,
                )
            sd_t = wpool.tile([128, JT, heads], F32, tag="sd")
            sc4 = sc_t[:, :J2, :].rearrange("p j (h s) -> p j h s", s=2)
            nc.vector.tensor_tensor(
                out=sd_t[:, :J2, :],
                in0=sc4[:, :, :, 0],
                in1=sc4[:, :, :, 1],
                op=ALU.subtract,
            )
            # alpha = exp(score) written straight into the payload
            nc.scalar.activation(
                ev_t[:, :J2, C : C + heads], sd_t[:, :J2, :], AF.Exp
            )
            # w = el * alpha
            nc.vector.tensor_tensor(
                out=ev_t[:, :J2, :C].rearrange("p j (h c) -> p j h c", h=heads),
                in0=el.rearrange("p j (h c) -> p j h c", h=heads),
                in1=ev_t[:, :J2, C : C + heads]
                .unsqueeze(3)
                .broadcast_to([128, J2, heads, ch]),
                op=ALU.mult,
            )

        # ---------------- self-loop subtiles (all blocks, batched)
        SLJ = JT  # process self-loop blocks in JT-sized batches
        for b0 in range(0, nblk, SLJ):
            nb2 = min(SLJ, nblk - b0)
            ev_t = wpool.tile([128, JT, W], BF16, tag="ev")
            edge_math(
                xlo_t[:, b0 : b0 + nb2, :], xrt_t[:, b0 : b0 + nb2, :], nb2, ev_t
            )
            for i in range(nb2):
                b = b0 + i
                bank, off = b // per_bank, (b % per_bank) * W
                nc.tensor.matmul(
                    out=pacc[bank][:, off : off + W],
                    lhsT=ident_t[:],
                    rhs=ev_t[:, i, :],
                    start=(b % per_bank == 0),
                    stop=False,
                )

        # ---------------- edge sweep
        ti = 0
        banks_done = []
        sub_of_tile = [t[3] for t in plan.tiles]
        cnt_reg = nc.gpsimd.alloc_register("cnt_reg")
        for ci, (chk, nsb) in enumerate(plan.calls):
            n_idx = nsb * P
            cols = n_idx // 16
            iel_t = ipool.tile([128, JCALL * 8], I16, tag="iel")
            nc.sync.dma_start(iel_t[:, :cols], idx_el_d[ci, :, :cols])
            el_t = gpool.tile([128, JCALL, 128], BF16, tag="el")
            src_tab = elt1 if chk else elt0
            if USE_CNT_REG:
                nc.gpsimd.reg_load(cnt_reg, cnt_t[0:1, ci : ci + 1])
                reg = cnt_reg
            else:
                reg = n_idx
            nc.gpsimd.dma_gather(
                el_t[:, :nsb, :], src_tab[:, :], iel_t[:, :cols], n_idx, reg,
                128, single_packet=False,
            )

            j0 = 0
            while j0 < nsb:
                J2 = min(JT, nsb - j0)
                sub0 = sub_of_tile[ti]
                el = el_t[:, j0 : j0 + J2, :C]
                ds_t = ipool.tile([128, JT], BF16, tag="ds")
                nc.sync.dma_start(ds_t[:, :J2], ds_d[ti, :, :J2])
                dsr_t = ipool.tile([128, JT * 128], BF16, tag="dsr")
                nc.sync.dma_start(
                    dsr_t[:, : J2 * 128],
                    dsf_d[ti, 0:1, : J2 * 128].to_broadcast([128, J2 * 128]),
                )

                # scatter one-hot [e, j, d] = (iota[d] == ds[e, j])
                oh_t = wpool.tile([128, JT, 128], BF16, tag="oh")
                nc.vector.tensor_tensor(
                    out=oh_t[:, :J2, :],
                    in0=iota_t[:].unsqueeze(1).broadcast_to([128, J2, 128]),
                    in1=ds_t[:, :J2].unsqueeze(2).broadcast_to([128, J2, 128]),
                    op=ALU.is_equal,
                )
                # expansion one-hot [d, e] = (dsr[d, e] == d)
                ohT_t = wpool.tile([128, JT * 128], BF16, tag="ohT")
                nc.vector.tensor_tensor(
                    out=ohT_t[:, : J2 * 128],
                    in0=dsr_t[:, : J2 * 128],
                    in1=iotapr_t[:, : J2 * 128],
                    op=ALU.is_equal,
                )

                # expansion: xr_d[e, :] in PSUM via onehotT.T @ xr_blk
                xrd_t = wpool.tile([128, JT, C], BF16, tag="xrd")
                for g0 in range(0, J2, EXG):
                    gn = min(EXG, J2 - g0)
                    exg_t = expool.tile([128, EXG * C], F32, tag="exg", name="exg")
                    for j in range(g0, g0 + gn):
                        nc.tensor.matmul(
                            out=exg_t[:, (j - g0) * C : (j - g0 + 1) * C],
                            lhsT=ohT_t[:, j * 128 : (j + 1) * 128],
                            rhs=xrt_t[:, plan.sub_block[sub0 + j], :],
                            start=True,
                            stop=True,
                        )
                    nc.scalar.activation(
                        xrd_t[:, g0 : g0 + gn, :],
                        exg_t[:, : gn * C].rearrange("p (g c) -> p g c", c=C),
                        AF.Copy,
                    )

                ev_t = wpool.tile([128, JT, W], BF16, tag="ev")
                edge_math(el, xrd_t[:, :J2, :], J2, ev_t)

                stopped = []
                for j in range(J2):
                    sb = sub0 + j
                    b = plan.sub_block[sb]
                    bank, off = b // per_bank, (b % per_bank) * W
                    sp = plan.sub_stop[sb] and (
                        b % per_bank == per_bank - 1 or b == nblk - 1
                    )
                    nc.tensor.matmul(
                        out=pacc[bank][:, off : off + W],
                        lhsT=oh_t[:, j, :],
                        rhs=ev_t[:, j, :],
                        start=False,
                        stop=sp,
                    )
                    if sp:
                        stopped.append(bank)
                for bank in stopped:
                    if bank not in banks_done:
                        banks_done.append(bank)
                        do_norm(bank)
                ti += 1
                j0 += J2

        for k in range(n_banks):
            if k not in banks_done:
                do_norm(k)

    return nc


# ============================================================== entry point
_CACHE = {}


def make_in_maps(plan, heads, ch, perm, scale, xl2, xr2, bias, n_graphs):
    C2 = heads * ch
    import ml_dtypes

    iota128 = np.tile(np.arange(128, dtype=np.float32), (128, 1))
    iota_bf = iota128.astype(ml_dtypes.bfloat16)
    ident = np.eye(128, dtype=np.float32).astype(ml_dtypes.bfloat16)
    iota_p = np.arange(128, dtype=np.float32).reshape(128, 1)
    iota_pr = np.tile(iota_p, (1, JT * 128)).astype(ml_dtypes.bfloat16)

    bias = np.asarray(bias, np.float64)
    b2 = bias[perm]
    ia2 = 1.0 / scale
    b_rep = np.tile(b2.astype(np.float32).reshape(1, C2), (128, 1))
    ia_rep = np.tile(ia2.astype(np.float32).reshape(1, C2), (128, 1))

    in_maps = []
    for c in range(plan.n_cores):
        t0, t1, xrt, xlo = plan.make_tables(xl2, xr2, c)
        m = {
            "elt0": t0,
            "elt1": t1,
            "xrt": xrt,
            "xlo": xlo,
            "iota128": iota_bf,
            "ident": ident,
            "iota_pr": iota_pr,
            "b_rep": b_rep,
            "inv_att": ia_rep,
            "idx_el": plan.core_el_idx[c],
            "cnt": plan.core_cnt[c],
            "ds": plan.core_ds[c],
            "dsf": plan.core_dsf[c],
        }
        in_maps.append(m)
    return in_maps


LAST_RESULTS = []  # BassKernelResults per layer launch (for test harness)


def _maybe_install_ntff_hook():
    """BASS_TRACE=1 needs antenv.axon_hooks, which this container lacks;
    synthesize it from the ctypes hook in trn_agent_boot."""
    if not os.environ.get("BASS_TRACE"):
        return
    import types

    if "antenv.axon_hooks" in sys.modules:
        return
    try:
        if "/root/.axon_site" not in sys.path:
            sys.path.insert(0, "/root/.axon_site")
        from trn_agent_boot.trn_boot import _ntff_profile_via_ctypes

        hook = _ntff_profile_via_ctypes("/opt/axon/libaxon_pjrt.so")
        m = types.ModuleType("antenv.axon_hooks")
        m.get_axon_ntff_profile_hook = lambda: hook
        sys.modules["antenv.axon_hooks"] = m
    except Exception:
        pass


def run_layer(plan, key, heads, ch, xl, xr, att, bias, n_graphs=N_GRAPHS):
    """Run one layer on HW; returns full h [n_nodes, heads*ch] (un-permuted,
    pre-concat of per-core shards)."""
    from concourse.bass_utils import run_bass_kernel_spmd

    _maybe_install_ntff_hook()

    perm, scale, glens = fold_att(att, heads, ch)
    xl = np.asarray(xl, np.float64)
    xr = np.asarray(xr, np.float64)
    xl2 = (xl[:, perm] * scale).astype(np.float32)
    xr2 = (xr[:, perm] * scale).astype(np.float32)

    cache_key = (key, heads, tuple(glens))
    if cache_key not in _CACHE:
        nc = build_layer(plan, heads, ch, glens, False, n_graphs=n_graphs)
        if not nc.is_finalized():
            nc.finalize()
        _CACHE[cache_key] = nc
    nc = _CACHE[cache_key]
    in_maps = make_in_maps(plan, heads, ch, perm, scale, xl2, xr2, bias, n_graphs)
    res = run_bass_kernel_spmd(nc, in_maps, core_ids=list(range(plan.n_cores)))
    LAST_RESULTS.append(res)
    h2 = np.concatenate(
        [res.results[c]["h_out"][: plan.npc] for c in range(plan.n_cores)], axis=0
    )
    # un-permute folded channels back to [heads*ch]
    h = np.zeros((plan.n_nodes, heads * ch), np.float32)
    h[:, perm] = h2
    return h


def gat_forward(
    x, edge_index, batch, Wl1, Wr1, att1, b1, Wl2, Wr2, att2, b2, lin_w, lin_b,
    plan, heads1=HEADS, ch1=DIM_H, ch2=DIM_OUT, n_graphs=N_GRAPHS,
):
    x = np.asarray(x, np.float32)
    xl1 = x @ np.asarray(Wl1, np.float32)
    xr1 = x @ np.asarray(Wr1, np.float32)
    h = run_layer(plan, "l1", heads1, ch1, xl1, xr1, np.asarray(att1), b1)

    hl2 = h @ np.asarray(Wl2, np.float32)
    hr2 = h @ np.asarray(Wr2, np.float32)
    h2 = run_layer(plan, "l2", 1, ch2, hl2, hr2, np.asarray(att2), b2)

    bidx = np.asarray(batch, np.int64)
    sums = np.zeros((n_graphs, ch2), np.float32)
    np.add.at(sums, bidx, h2)
    cnts = np.bincount(bidx, minlength=n_graphs).astype(np.float32)
    pooled = sums / np.maximum(cnts, 1.0)[:, None]
    out = pooled @ np.asarray(lin_w, np.float32) + np.asarray(lin_b, np.float32)
    return out.astype(np.float32)


def kernel(x, edge_index, batch, Wl1, Wr1, att1, b1, Wl2, Wr2, att2, b2, lin_w, lin_b):
    plan = GraphPlan(np.asarray(edge_index), N_NODES, N_CORES)
    return gat_forward(
        x, edge_index, batch, Wl1, Wr1, att1, b1, Wl2, Wr2, att2, b2, lin_w, lin_b,
        plan,
    )


# revision 24
# speedup vs baseline: 1.0383x; 1.0383x over previous
"""GATv2 (2-layer) + global mean pool on 8 Trainium2 NeuronCores.

Strategy (per core): nodes are partitioned into 8 contiguous blocks of 6250
dst nodes.  Each core processes all edges whose dst lands in its block.
Edges are host-sorted by (src-chunk, dst-block-of-128) and padded so every
(chunk, block) group has a fixed number of 128-edge subtiles -> the Bass
instruction stream is identical on all 8 cores (SPMD), only data differs.

Device per layer (bound by Q7/SWDGE gather descriptor generation at
~8ns/edge; everything else overlaps under it):
  - el = table[src] via dma_gather from a bf16 table whose rows are padded
    to 128 cols so each descriptor is the 256B minimum.  One gather call per
    (chunk, dst-block) group; pad slots point at a zero dummy row.  (The
    num_idxs_reg / trailing -1 trim variants that would skip pad descriptors
    are implemented behind GAT_CNT_REG / GAT_NEG_TRIM but default OFF: on
    HW both wedge the device, likely a DMA-semaphore hang when an SDMA
    engine receives no descriptors.)
  - self-loop edges never enter the gather: per block they are one subtile
    whose el/xr are direct reads of small resident tables and whose scatter
    one-hot is the identity; all 49 are batched into a handful of DVE ops
    emitted before the edge sweep (they fill the desc-gen startup bubble and
    carry the PSUM bank start=True flags).
  - |att| is folded into the xl/xr tables on the host with channels permuted
    into exact (head, sign-of-att) groups: the score becomes
    sum_c lrelu(z')[group +] - sum_c lrelu(z')[group -] (one small reduce
    per group), killing the per-edge r*att multiply on DVE.  The fold is
    undone in the per-bank normalize by a per-channel 1/|att| multiply (and
    the host un-permutes h_out columns).
  - xr_d expansion on the tensor engine: onehotT matmul'd with the 128-row
    xr block (xrt stays resident in SBUF) -> PSUM, drained by ACT.
  - bf16 edge math on DVE: z = el + xr_d ; r = leaky_relu(z); score =
    signed group reduce; alpha' = exp(score) written by ACT directly into
    the scatter payload ev[:, C':C'+h].
  - w = alpha' * el (bf16), scattered to dst via one-hot matmuls
    accumulated in PSUM (49 block accumulators packed 7-per-bank; start=True
    only on each bank's first matmul = its first self-loop subtile), then
    h = (num/(den+1e-16)) * inv|att| + b', ELU.  Each bank is normalized as
    soon as its last block stops accumulating.
Host does the projections (x@W) + the att fold/permute, the gather of h
between layers, the global mean pool, and the tiny [64,2] head.
"""

import os
import sys

import numpy as np


def _setup_paths():
    for p in (
        "/opt/trn_rl_repo",
        os.path.expanduser("~/.axon_site/_ro/trn_rl_repo"),
    ):
        if os.path.isdir(p) and p not in sys.path:
            sys.path.insert(0, p)


_setup_paths()

import concourse.bacc as bacc  # noqa: E402
import concourse.bass as bass  # noqa: E402
import concourse.mybir as mybir  # noqa: E402
import concourse.tile as tile  # noqa: E402

AF = mybir.ActivationFunctionType
ALU = mybir.AluOpType
F32 = mybir.dt.float32
I16 = mybir.dt.int16
I32 = mybir.dt.int32

# ---------------------------------------------------------------- model dims
N_NODES = 50000
N_EDGES = 1600000
N_GRAPHS = 64
DIM_IN, DIM_H, HEADS, DIM_OUT = 128, 32, 2, 64
NEG_SLOPE = 0.2

N_CORES = 8
USE_CNT_REG = os.environ.get("GAT_CNT_REG", "0") == "1"
NEG_TRIM = os.environ.get("GAT_NEG_TRIM", "0") == "1"
P = 128          # partitions / edges per subtile
BLK = 128        # dst nodes per block
JT = 16          # subtiles per compute tile
JCALL = 32       # max subtiles per gather call
GRP = 4
CHUNK = 32767    # src-index chunk size (int16 limit)
ATT_EPS = 1e-6


def cdiv(a, b):
    return (a + b - 1) // b


# ============================================================== host prep
def pack_idx(v, pad_to_cols=None):
    """Pack an int array into the dma_gather idx layout:
    idx i -> partition i%16 (replicated to all 8 groups of 16), col i//16."""
    n = len(v)
    assert n % 16 == 0
    cols = n // 16
    a = np.asarray(v, np.int16).reshape(cols, 16).T  # [16, cols]
    a = np.tile(a, (8, 1))  # [128, cols]
    if pad_to_cols is not None and cols < pad_to_cols:
        a = np.pad(a, ((0, 0), (0, pad_to_cols - cols)), constant_values=-1)
    return a


def fold_att(att, heads, ch):
    """Channel permutation + scale for the att fold (no padding).

    Returns (perm, scale, glens): perm[slot] = source channel, scale[slot] =
    |att[src]| clamped, slots laid out as [h0+, h0-, h1+, h1-, ...] with
    exact group lengths glens[2h] (so each head's ch columns stay
    contiguous and C2 == heads*ch)."""
    a = np.asarray(att, np.float64).reshape(heads, ch)
    perm, scale, glens = [], [], []
    for h in range(heads):
        pos = np.where(a[h] >= 0)[0]
        neg = np.where(a[h] < 0)[0]
        for idxs in (pos, neg):
            glens.append(len(idxs))
            for c in idxs:
                perm.append(h * ch + c)
                scale.append(max(abs(a[h, c]), ATT_EPS))
    return np.asarray(perm, np.int64), np.asarray(scale, np.float64), glens


class GraphPlan:
    """Core-uniform edge layout shared by both layers (self-loops excluded
    from the gather; handled as identity subtiles)."""

    def __init__(self, edge_index, n_nodes, n_cores, chunk=CHUNK):
        src = np.asarray(edge_index[0], np.int64)
        dst = np.asarray(edge_index[1], np.int64)
        self.n_nodes = n_nodes
        self.n_cores = n_cores
        self.chunk = chunk
        npc = n_nodes // n_cores
        self.npc = npc
        nblk = cdiv(npc, BLK)
        self.nblk = nblk
        self.rows0 = min(chunk, n_nodes) + 1   # chunk0 src rows + dummy
        self.rows1 = max(n_nodes - chunk, 0) + 1  # chunk1 src rows + dummy
        self.xr_rows = nblk * BLK              # block rows (pad rows zero)

        core = dst // npc
        per = {}
        for c in range(n_cores):
            m = core == c
            s, d = src[m], dst[m]
            dloc = d - c * npc
            b = dloc // BLK
            ch = (s >= chunk).astype(np.int64)
            key = ch * nblk + b
            order = np.argsort(key, kind="stable")
            s, dloc, key = s[order], dloc[order], key[order]
            per[c] = (s, dloc, key)
        # per-(chunk, block) subtile counts: max over cores (static shapes)
        Sb = np.ones((2, nblk), np.int64)
        cnts = np.zeros((n_cores, 2, nblk), np.int64)
        for c in range(n_cores):
            _, _, key = per[c]
            cnt = np.bincount(key, minlength=2 * nblk)
            cnts[c] = cnt.reshape(2, nblk)
            for ch in (0, 1):
                v = np.maximum(1, -(-cnt[ch * nblk : (ch + 1) * nblk] // P))
                Sb[ch] = np.maximum(Sb[ch], v)
        self.Sb = Sb
        self.cnts = cnts  # real edge count per (core, chunk, block)
        self.n_sub_total = int(Sb.sum())
        self.Smax = int(Sb.max())

        # subtile -> (block, stop) ; uniform across cores.  start flags live
        # on the self-loop subtiles now.
        self.sub_block = []
        self.sub_stop = []
        for ch in (0, 1):
            for b in range(nblk):
                nsb = int(Sb[ch][b])
                for j in range(nsb):
                    self.sub_block.append(b)
                    self.sub_stop.append(ch == 1 and j == nsb - 1)

        # gather calls: slabs of <= JCALL subtiles per chunk, greedily
        # packing whole (chunk, block) groups (pads stay dummy-row descs,
        # so calls may span block boundaries freely).
        self.calls = []  # (chunk, n_subtiles)
        for ch in (0, 1):
            acc = 0
            for b in range(nblk):
                take = int(Sb[ch][b])
                if acc and acc + take > JCALL:
                    self.calls.append((ch, acc))
                    acc = 0
                acc += take
            if acc:
                self.calls.append((ch, acc))
        self.n_calls = len(self.calls)

        # compute tiles: JT-slices within each call
        self.tiles = []  # (call_idx, j0, J2, sub_base)
        sub_base = 0
        for ci, (ch, nsb) in enumerate(self.calls):
            j0 = 0
            while j0 < nsb:
                J2 = min(JT, nsb - j0)
                self.tiles.append((ci, j0, J2, sub_base + j0))
                j0 += J2
            sub_base += nsb
        self.n_tiles = len(self.tiles)

        # per-core slot data
        import ml_dtypes

        BF = ml_dtypes.bfloat16
        self.core_el_idx = []   # [n_calls, 128, JCALL*8] int16
        self.core_cnt = []      # [1, n_calls] int32 (real edges per call)
        self.core_ds = []       # [n_tiles, 128, JT] bf16 (edge-major)
        self.core_dsf = []      # [n_tiles, 1, JT*128] bf16 (free-major)
        dummy0 = self.rows0 - 1
        dummy1 = self.rows1 - 1
        # padded-subtile start of each (chunk, block) group, in group order
        grp_start = {}
        off = 0
        for ch in (0, 1):
            for b in range(nblk):
                grp_start[(ch, b)] = off
                off += int(Sb[ch][b])
        call_start = {}
        off = 0
        for ci, (ch, nsb) in enumerate(self.calls):
            call_start[ci] = off
            off += nsb
        for c in range(n_cores):
            s, dloc, key = per[c]
            el_slots = np.full(self.n_sub_total * P, 0, np.int32)
            ds_slots = np.full(self.n_sub_total * P, 240.0, np.float32)
            cnt = np.bincount(key, minlength=2 * nblk)
            starts = np.concatenate([[0], np.cumsum(cnt)[:-1]])
            cnt_arr = np.zeros((1, self.n_calls), np.int32)
            for ch in (0, 1):
                for b in range(nblk):
                    k = ch * nblk + b
                    n = int(cnt[k])
                    a0 = int(starts[k])
                    off = grp_start[(ch, b)] * P
                    sl = slice(off, off + n)
                    ss = s[a0 : a0 + n]
                    el_slots[sl] = ss - (chunk if ch else 0)
                    ds_slots[sl] = (dloc[a0 : a0 + n] % BLK).astype(np.float32)
                    # pad slots point at the zero dummy row (GAT_NEG_TRIM=1
                    # trailing -1s wedge the device; see module docstring)
                    pad = slice(off + n, off + int(Sb[ch][b]) * P)
                    el_slots[pad] = (
                        -1 if NEG_TRIM else (dummy1 if ch else dummy0)
                    )
            self.core_cnt.append(cnt_arr)
            el_idx = np.full((self.n_calls, 128, JCALL * 8), -1, np.int16)
            for ci, (ch, nsb) in enumerate(self.calls):
                n = nsb * P
                base = call_start[ci] * P
                el_idx[ci, :, : n // 16] = pack_idx(el_slots[base : base + n])
            self.core_el_idx.append(el_idx)
            ds = np.full((self.n_tiles, 128, JT), 240.0, np.float32)
            dsf = np.full((self.n_tiles, 1, JT * 128), 240.0, np.float32)
            for ti, (ci, j0, J2, sb) in enumerate(self.tiles):
                blkv = ds_slots[sb * P : (sb + J2) * P].reshape(J2, P)
                ds[ti, :, :J2] = blkv.T
                dsf[ti, 0, : J2 * 128] = blkv.reshape(-1)
            self.core_ds.append(ds.astype(BF))
            self.core_dsf.append(dsf.astype(BF))

    def make_tables(self, xl2, xr2, core):
        """el chunk tables (bf16, rows padded to 128 cols), xr block table
        and own-range xl table for one core.  xl2/xr2 are the FOLDED
        [n_nodes, C2] node features."""
        import ml_dtypes

        BF = ml_dtypes.bfloat16
        C2 = xl2.shape[1]
        ck = self.chunk
        t0 = np.zeros((self.rows0, 128), BF)
        t0[: min(ck, self.n_nodes), :C2] = xl2[:ck].astype(BF)
        t1 = np.zeros((self.rows1, 128), BF)
        t1[: max(self.n_nodes - ck, 0), :C2] = xl2[ck:].astype(BF)
        lo = core * self.npc
        hi = lo + self.npc
        xrt = np.zeros((self.xr_rows, C2), np.float32)
        xrt[: self.npc] = xr2[lo:hi]
        xlo = np.zeros((self.xr_rows, C2), np.float32)
        xlo[: self.npc] = xl2[lo:hi]
        # rearrange to [128, nblk*C2] (partition-major blocks)
        xrt_r = xrt.reshape(self.nblk, BLK, C2).transpose(1, 0, 2).reshape(BLK, -1)
        xlo_r = xlo.reshape(self.nblk, BLK, C2).transpose(1, 0, 2).reshape(BLK, -1)
        return t0, t1, xrt_r.astype(BF), xlo_r.astype(BF)


# ============================================================== bass builder
def build_layer(plan: GraphPlan, heads, ch, glens, do_pool, n_graphs=N_GRAPHS):
    """One GATv2 layer for one core (SPMD across 8).  glens: the 2h exact
    (head, sign) group lengths of the permuted channel layout."""
    BF16 = mybir.dt.bfloat16
    C = heads * ch               # channel count (permuted, unpadded)
    W = C + heads                # psum accum row: [w-sums | exp-sums]
    assert sum(glens) == C and len(glens) == 2 * heads
    goff = np.concatenate([[0], np.cumsum(glens)]).astype(int)
    nblk = plan.nblk
    per_bank = 7
    n_banks = cdiv(nblk, per_bank)
    assert per_bank * W <= 512 and n_banks <= 7, (W, n_banks)
    EXG = 8

    nc = bacc.Bacc()
    elt0 = nc.dram_tensor("elt0", [plan.rows0, 128], BF16, kind="ExternalInput")
    elt1 = nc.dram_tensor("elt1", [plan.rows1, 128], BF16, kind="ExternalInput")
    xrt_d = nc.dram_tensor("xrt", [128, nblk * C], BF16, kind="ExternalInput")
    xlo_d = nc.dram_tensor("xlo", [128, nblk * C], BF16, kind="ExternalInput")
    iota_d = nc.dram_tensor("iota128", [128, 128], BF16, kind="ExternalInput")
    ident_d = nc.dram_tensor("ident", [128, 128], BF16, kind="ExternalInput")
    iotapr_d = nc.dram_tensor(
        "iota_pr", [128, JT * 128], BF16, kind="ExternalInput"
    )
    b_d = nc.dram_tensor("b_rep", [128, C], F32, kind="ExternalInput")
    ia_d = nc.dram_tensor("inv_att", [128, C], F32, kind="ExternalInput")
    idx_el_d = nc.dram_tensor(
        "idx_el", [plan.n_calls, 128, JCALL * 8], I16, kind="ExternalInput"
    )
    cnt_d = nc.dram_tensor("cnt", [1, plan.n_calls], I32, kind="ExternalInput")
    ds_d = nc.dram_tensor("ds", [plan.n_tiles, 128, JT], BF16, kind="ExternalInput")
    dsf_d = nc.dram_tensor(
        "dsf", [plan.n_tiles, 1, JT * 128], BF16, kind="ExternalInput"
    )
    h_out = nc.dram_tensor("h_out", [nblk * BLK, C], F32, kind="ExternalOutput")

    from contextlib import ExitStack

    with tile.TileContext(nc) as tc, ExitStack() as ctx:
        cpool = ctx.enter_context(tc.tile_pool(name="consts", bufs=1))
        gpool = ctx.enter_context(tc.tile_pool(name="gather", bufs=6))
        ipool = ctx.enter_context(tc.tile_pool(name="idx", bufs=12))
        wpool = ctx.enter_context(tc.tile_pool(name="work", bufs=3))
        npool = ctx.enter_context(tc.tile_pool(name="norm", bufs=2))
        pspool = ctx.enter_context(tc.tile_pool(name="ps", bufs=1, space="PSUM"))
        expool = ctx.enter_context(tc.tile_pool(name="exps", bufs=1, space="PSUM"))

        iota_t = cpool.tile([128, 128], BF16, tag="iota")
        nc.sync.dma_start(iota_t[:], iota_d[:, :])
        ident_t = cpool.tile([128, 128], BF16, tag="ident")
        nc.sync.dma_start(ident_t[:], ident_d[:, :])
        iotapr_t = cpool.tile([128, JT * 128], BF16, tag="iotapr")
        nc.sync.dma_start(iotapr_t[:], iotapr_d[:, :])
        b_t = cpool.tile([128, C], F32, tag="bias")
        nc.sync.dma_start(b_t[:], b_d[:, :])
        ia_t = cpool.tile([128, C], F32, tag="invatt")
        nc.sync.dma_start(ia_t[:], ia_d[:, :])
        xrt_t = cpool.tile([128, nblk, C], BF16, tag="xrt")
        nc.sync.dma_start(
            xrt_t[:], xrt_d[:, :].rearrange("p (n c) -> p n c", c=C)
        )
        xlo_t = cpool.tile([128, nblk, C], BF16, tag="xlo")
        nc.sync.dma_start(
            xlo_t[:], xlo_d[:, :].rearrange("p (n c) -> p n c", c=C)
        )
        cnt_t = cpool.tile([1, plan.n_calls], I32, tag="cnt")
        nc.sync.dma_start(cnt_t[:], cnt_d[:, :])

        pacc = [
            pspool.tile([128, per_bank * W], F32, tag=f"pacc{k}", name=f"pacc{k}")
            for k in range(n_banks)
        ]

        # ---------------- normalize + ELU (per bank)
        def do_norm(k):
            nb = min(per_bank, nblk - k * per_bank)
            acc = pacc[k][:, : nb * W].rearrange("p (n w) -> p n w", w=W)
            den_t = npool.tile([128, per_bank * heads], F32, tag="den")
            den = den_t[:, : nb * heads].rearrange("p (n h) -> p n h", h=heads)
            nc.vector.tensor_scalar(den, acc[:, :, C : C + heads], 1e-16, None, ALU.add)
            rec_t = npool.tile([128, per_bank * heads], F32, tag="rec")
            rec = rec_t[:, : nb * heads].rearrange("p (n h) -> p n h", h=heads)
            nc.vector.reciprocal(rec, den)
            h_t = npool.tile([128, per_bank * C], F32, tag="h")
            hv = h_t[:, : nb * C].rearrange("p (n c) -> p n c", c=C)
            nc.vector.tensor_tensor(
                out=hv.rearrange("p n (h c) -> p n h c", h=heads),
                in0=acc[:, :, :C].rearrange("p n (h c) -> p n h c", h=heads),
                in1=rec.unsqueeze(3).broadcast_to([128, nb, heads, ch]),
                op=ALU.mult,
            )
            # undo the |att| fold, then + bias
            nc.vector.tensor_tensor(
                out=hv,
                in0=hv,
                in1=ia_t[:].unsqueeze(1).broadcast_to([128, nb, C]),
                op=ALU.mult,
            )
            nc.vector.tensor_tensor(
                out=hv,
                in0=hv,
                in1=b_t[:].unsqueeze(1).broadcast_to([128, nb, C]),
                op=ALU.add,
            )
            # ELU = relu(x) + exp(min(x,0)) - 1
            re_t = npool.tile([128, per_bank * C], F32, tag="re")
            nc.vector.tensor_scalar_max(re_t[:, : nb * C], h_t[:, : nb * C], 0.0)
            mn_t = npool.tile([128, per_bank * C], F32, tag="mn")
            nc.vector.tensor_scalar_min(mn_t[:, : nb * C], h_t[:, : nb * C], 0.0)
            nc.scalar.activation(mn_t[:, : nb * C], mn_t[:, : nb * C], AF.Exp)
            nc.vector.tensor_tensor(
                out=h_t[:, : nb * C],
                in0=re_t[:, : nb * C],
                in1=mn_t[:, : nb * C],
                op=ALU.add,
            )
            nc.vector.tensor_scalar_add(h_t[:, : nb * C], h_t[:, : nb * C], -1.0)
            for i in range(nb):
                b = k * per_bank + i
                nc.sync.dma_start(h_out[b * BLK : (b + 1) * BLK, :], hv[:, i, :])

        # ---------------- edge-math helper (shared by self-loop + gather)
        def edge_math(el, xrd, J2, ev_t):
            """el, xrd: [128, J2, C] bf16 views.  Fills ev_t[:, :J2, :W]."""
            z_t = wpool.tile([128, JT, C], BF16, tag="z")
            nc.vector.tensor_tensor(
                out=z_t[:, :J2, :], in0=el, in1=xrd, op=ALU.add
            )
            r_t = wpool.tile([128, JT, C], BF16, tag="r")
            nc.vector.scalar_tensor_tensor(
                out=r_t[:, :J2, :],
                in0=z_t[:, :J2, :],
                scalar=NEG_SLOPE,
                in1=z_t[:, :J2, :],
                op0=ALU.mult,
                op1=ALU.max,
            )
            # signed group reduce: exact per-(head, sign) partial sums
            sc_t = wpool.tile([128, JT, 2 * heads], F32, tag="sc")
            for g in range(2 * heads):
                if glens[g] == 0:
                    nc.vector.memset(sc_t[:, :J2, g : g + 1], 0.0)
                    continue
                nc.vector.reduce_sum(
                    out=sc_t[:, :J2, g : g + 1],
                    in_=r_t[:, :J2, goff[g] : goff[g + 1]].unsqueeze(2),
                    axis=mybir.AxisListType.X,
                )
            sd_t = wpool.tile([128, JT, heads], F32, tag="sd")
            sc4 = sc_t[:, :J2, :].rearrange("p j (h s) -> p j h s", s=2)
            nc.vector.tensor_tensor(
                out=sd_t[:, :J2, :],
                in0=sc4[:, :, :, 0],
                in1=sc4[:, :, :, 1],
                op=ALU.subtract,
            )
            # alpha = exp(score) written straight into the payload
            nc.scalar.activation(
                ev_t[:, :J2, C : C + heads], sd_t[:, :J2, :], AF.Exp
            )
            # w = el * alpha
            nc.vector.tensor_tensor(
                out=ev_t[:, :J2, :C].rearrange("p j (h c) -> p j h c", h=heads),
                in0=el.rearrange("p j (h c) -> p j h c", h=heads),
                in1=ev_t[:, :J2, C : C + heads]
                .unsqueeze(3)
                .broadcast_to([128, J2, heads, ch]),
                op=ALU.mult,
            )

        # ---------------- self-loop subtiles (all blocks, batched)
        SLJ = JT  # process self-loop blocks in JT-sized batches
        for b0 in range(0, nblk, SLJ):
            nb2 = min(SLJ, nblk - b0)
            ev_t = wpool.tile([128, JT, W], BF16, tag="ev")
            edge_math(
                xlo_t[:, b0 : b0 + nb2, :], xrt_t[:, b0 : b0 + nb2, :], nb2, ev_t
            )
            for i in range(nb2):
                b = b0 + i
                bank, off = b // per_bank, (b % per_bank) * W
                nc.tensor.matmul(
                    out=pacc[bank][:, off : off + W],
                    lhsT=ident_t[:],
                    rhs=ev_t[:, i, :],
                    start=(b % per_bank == 0),
                    stop=False,
                )

        # ---------------- edge sweep
        ti = 0
        banks_done = []
        sub_of_tile = [t[3] for t in plan.tiles]
        cnt_reg = nc.gpsimd.alloc_register("cnt_reg")
        for ci, (chk, nsb) in enumerate(plan.calls):
            n_idx = nsb * P
            cols = n_idx // 16
            iel_t = ipool.tile([128, JCALL * 8], I16, tag="iel")
            nc.sync.dma_start(iel_t[:, :cols], idx_el_d[ci, :, :cols])
            el_t = gpool.tile([128, JCALL, 128], BF16, tag="el")
            src_tab = elt1 if chk else elt0
            if USE_CNT_REG:
                nc.gpsimd.reg_load(cnt_reg, cnt_t[0:1, ci : ci + 1])
                reg = cnt_reg
            else:
                reg = n_idx
            nc.gpsimd.dma_gather(
                el_t[:, :nsb, :], src_tab[:, :], iel_t[:, :cols], n_idx, reg,
                128, single_packet=False,
            )

            j0 = 0
            while j0 < nsb:
                J2 = min(JT, nsb - j0)
                sub0 = sub_of_tile[ti]
                el = el_t[:, j0 : j0 + J2, :C]
                ds_t = ipool.tile([128, JT], BF16, tag="ds")
                nc.sync.dma_start(ds_t[:, :J2], ds_d[ti, :, :J2])
                dsr_t = ipool.tile([128, JT * 128], BF16, tag="dsr")
                nc.sync.dma_start(
                    dsr_t[:, : J2 * 128],
                    dsf_d[ti, 0:1, : J2 * 128].to_broadcast([128, J2 * 128]),
                )

                # scatter one-hot [e, j, d] = (iota[d] == ds[e, j])
                oh_t = wpool.tile([128, JT, 128], BF16, tag="oh")
                nc.vector.tensor_tensor(
                    out=oh_t[:, :J2, :],
                    in0=iota_t[:].unsqueeze(1).broadcast_to([128, J2, 128]),
                    in1=ds_t[:, :J2].unsqueeze(2).broadcast_to([128, J2, 128]),
                    op=ALU.is_equal,
                )
                # expansion one-hot [d, e] = (dsr[d, e] == d)
                ohT_t = wpool.tile([128, JT * 128], BF16, tag="ohT")
                nc.vector.tensor_tensor(
                    out=ohT_t[:, : J2 * 128],
                    in0=dsr_t[:, : J2 * 128],
                    in1=iotapr_t[:, : J2 * 128],
                    op=ALU.is_equal,
                )

                # expansion: xr_d[e, :] in PSUM via onehotT.T @ xr_blk
                xrd_t = wpool.tile([128, JT, C], BF16, tag="xrd")
                for g0 in range(0, J2, EXG):
                    gn = min(EXG, J2 - g0)
                    exg_t = expool.tile([128, EXG * C], F32, tag="exg", name="exg")
                    for j in range(g0, g0 + gn):
                        nc.tensor.matmul(
                            out=exg_t[:, (j - g0) * C : (j - g0 + 1) * C],
                            lhsT=ohT_t[:, j * 128 : (j + 1) * 128],
                            rhs=xrt_t[:, plan.sub_block[sub0 + j], :],
                            start=True,
                            stop=True,
                        )
                    nc.scalar.activation(
                        xrd_t[:, g0 : g0 + gn, :],
                        exg_t[:, : gn * C].rearrange("p (g c) -> p g c", c=C),
                        AF.Copy,
                    )

                ev_t = wpool.tile([128, JT, W], BF16, tag="ev")
                edge_math(el, xrd_t[:, :J2, :], J2, ev_t)

                stopped = []
                for j in range(J2):
                    sb = sub0 + j
                    b = plan.sub_block[sb]
                    bank, off = b // per_bank, (b % per_bank) * W
                    sp = plan.sub_stop[sb] and (
                        b % per_bank == per_bank - 1 or b == nblk - 1
                    )
                    nc.tensor.matmul(
                        out=pacc[bank][:, off : off + W],
                        lhsT=oh_t[:, j, :],
                        rhs=ev_t[:, j, :],
                        start=False,
                        stop=sp,
                    )
                    if sp:
                        stopped.append(bank)
                for bank in stopped:
                    if bank not in banks_done:
                        banks_done.append(bank)
                        do_norm(bank)
                ti += 1
                j0 += J2

        for k in range(n_banks):
            if k not in banks_done:
                do_norm(k)

    return nc


# ============================================================== entry point
_CACHE = {}


def make_in_maps(plan, heads, ch, perm, scale, xl2, xr2, bias, n_graphs):
    C2 = heads * ch
    import ml_dtypes

    iota128 = np.tile(np.arange(128, dtype=np.float32), (128, 1))
    iota_bf = iota128.astype(ml_dtypes.bfloat16)
    ident = np.eye(128, dtype=np.float32).astype(ml_dtypes.bfloat16)
    iota_p = np.arange(128, dtype=np.float32).reshape(128, 1)
    iota_pr = np.tile(iota_p, (1, JT * 128)).astype(ml_dtypes.bfloat16)

    bias = np.asarray(bias, np.float64)
    b2 = bias[perm]
    ia2 = 1.0 / scale
    b_rep = np.tile(b2.astype(np.float32).reshape(1, C2), (128, 1))
    ia_rep = np.tile(ia2.astype(np.float32).reshape(1, C2), (128, 1))

    in_maps = []
    for c in range(plan.n_cores):
        t0, t1, xrt, xlo = plan.make_tables(xl2, xr2, c)
        m = {
            "elt0": t0,
            "elt1": t1,
            "xrt": xrt,
            "xlo": xlo,
            "iota128": iota_bf,
            "ident": ident,
            "iota_pr": iota_pr,
            "b_rep": b_rep,
            "inv_att": ia_rep,
            "idx_el": plan.core_el_idx[c],
            "cnt": plan.core_cnt[c],
            "ds": plan.core_ds[c],
            "dsf": plan.core_dsf[c],
        }
        in_maps.append(m)
    return in_maps


LAST_RESULTS = []  # BassKernelResults per layer launch (for test harness)


def _maybe_install_ntff_hook():
    """BASS_TRACE=1 needs antenv.axon_hooks, which this container lacks;
    synthesize it from the ctypes hook in trn_agent_boot."""
    if not os.environ.get("BASS_TRACE"):
        return
    import types

    if "antenv.axon_hooks" in sys.modules:
        return
    try:
        if "/root/.axon_site" not in sys.path:
            sys.path.insert(0, "/root/.axon_site")
        from trn_agent_boot.trn_boot import _ntff_profile_via_ctypes

        hook = _ntff_profile_via_ctypes("/opt/axon/libaxon_pjrt.so")
        m = types.ModuleType("antenv.axon_hooks")
        m.get_axon_ntff_profile_hook = lambda: hook
        sys.modules["antenv.axon_hooks"] = m
    except Exception:
        pass


def run_layer(plan, key, heads, ch, xl, xr, att, bias, n_graphs=N_GRAPHS):
    """Run one layer on HW; returns full h [n_nodes, heads*ch] (un-permuted,
    pre-concat of per-core shards)."""
    from concourse.bass_utils import run_bass_kernel_spmd

    _maybe_install_ntff_hook()

    perm, scale, glens = fold_att(att, heads, ch)
    xl = np.asarray(xl, np.float64)
    xr = np.asarray(xr, np.float64)
    xl2 = (xl[:, perm] * scale).astype(np.float32)
    xr2 = (xr[:, perm] * scale).astype(np.float32)

    cache_key = (key, heads, tuple(glens))
    if cache_key not in _CACHE:
        nc = build_layer(plan, heads, ch, glens, False, n_graphs=n_graphs)
        if not nc.is_finalized():
            nc.finalize()
        _CACHE[cache_key] = nc
    nc = _CACHE[cache_key]
    in_maps = make_in_maps(plan, heads, ch, perm, scale, xl2, xr2, bias, n_graphs)
    res = run_bass_kernel_spmd(nc, in_maps, core_ids=list(range(plan.n_cores)))
    LAST_RESULTS.append(res)
    h2 = np.concatenate(
        [res.results[c]["h_out"][: plan.npc] for c in range(plan.n_cores)], axis=0
    )
    # un-permute folded channels back to [heads*ch]
    h = np.zeros((plan.n_nodes, heads * ch), np.float32)
    h[:, perm] = h2
    return h


def gat_forward(
    x, edge_index, batch, Wl1, Wr1, att1, b1, Wl2, Wr2, att2, b2, lin_w, lin_b,
    plan, heads1=HEADS, ch1=DIM_H, ch2=DIM_OUT, n_graphs=N_GRAPHS,
):
    x = np.asarray(x, np.float32)
    xl1 = x @ np.asarray(Wl1, np.float32)
    xr1 = x @ np.asarray(Wr1, np.float32)
    h = run_layer(plan, "l1", heads1, ch1, xl1, xr1, np.asarray(att1), b1)

    hl2 = h @ np.asarray(Wl2, np.float32)
    hr2 = h @ np.asarray(Wr2, np.float32)
    h2 = run_layer(plan, "l2", 1, ch2, hl2, hr2, np.asarray(att2), b2)

    bidx = np.asarray(batch, np.int64)
    sums = np.zeros((n_graphs, ch2), np.float32)
    np.add.at(sums, bidx, h2)
    cnts = np.bincount(bidx, minlength=n_graphs).astype(np.float32)
    pooled = sums / np.maximum(cnts, 1.0)[:, None]
    out = pooled @ np.asarray(lin_w, np.float32) + np.asarray(lin_b, np.float32)
    return out.astype(np.float32)


def kernel(x, edge_index, batch, Wl1, Wr1, att1, b1, Wl2, Wr2, att2, b2, lin_w, lin_b):
    plan = GraphPlan(np.asarray(edge_index), N_NODES, N_CORES)
    return gat_forward(
        x, edge_index, batch, Wl1, Wr1, att1, b1, Wl2, Wr2, att2, b2, lin_w, lin_b,
        plan,
    )
